# revision 20
# baseline (speedup 1.0000x reference)
"""Multi-head attention Trainium2 Bass kernel.

Problem: B=4, T=2048, D=1024, H=16 heads of dim 64 (fp32).
  qkv = x @ Wqkv.T + bqkv ; per-head attention ; out @ Wo.T + bo

Sharding: 8 cores = 4 batches x 2 head-groups of 8 heads.  Each core
computes its batch's attention for its 8 heads plus the out-projection
restricted to its heads' columns (partial sum); the host adds the two
partial outputs per batch (the "all-reduce") and transposes back.

All device matmuls run in float32r (fp22 mantissa truncation, full PE
rate at N=512).  Layouts are chosen so no on-device transposes are
needed: the host feeds c-major (transposed) activations/weights, and
attention uses the S^T = K@Q^T orientation so softmax denominators come
out of a ones-column in the PV matmul.

Per-core layout:
  xT    [1024, 2048] x[b]^T (c-major)
  wqkT  [1024, 4, 256] per head-pack p (2 heads): Q-pack cols | K-pack cols
  bqk   [128, 4, 2]   per-partition q/k biases matching the pack layout
  wvT   [1024, 512]   V projection (c-major), heads side by side
  bv    [128, 4]      v-bias laid out like the packed PV output partitions
  woT   [512, 1024]   Wo columns for this head-group, c_in-major
  bo    [128, 8]      out bias (zeros on head-group-1 cores so it is
                      added exactly once per batch after the host sum)
  yT    [1024, 2048]  output y^T partial (f32)
"""

import sys

sys.path.insert(0, "/opt/trn_rl_repo")

import numpy as np

import concourse.bass as bass  # noqa: F401  (import keeps bass registered)
from concourse import bacc
import concourse.mybir as mybir
import concourse.tile as tile
from concourse.bass_utils import run_bass_kernel_spmd

B, T, D = 4, 2048, 1024
H, HD = 16, 64
P = 128
FP32 = mybir.dt.float32
FP32R = mybir.dt.float32r
AF = mybir.ActivationFunctionType
OP = mybir.AluOpType

N_CORES = 8
HPC = 8          # heads per core
NPACK = 4        # head pairs per core
CT = D // P      # 8 contraction tiles over D
KT = T // P      # 16 key tiles
QH = 2           # q halves of 1024
QHW = T // QH    # 1024
SCALE = HD ** -0.5


def build_nc(reps: int = 1, variant: str = "base", dyn: bool = False):
    nc = bacc.Bacc(None, target_bir_lowering=False, debug=False)

    xT_d = nc.dram_tensor("xT", [D, T], FP32R, kind="ExternalInput")
    wqkT_d = nc.dram_tensor("wqkT", [D, NPACK, 256], FP32R, kind="ExternalInput")
    bqk_d = nc.dram_tensor("bqk", [P, NPACK, 2], FP32, kind="ExternalInput")
    wvT_d = nc.dram_tensor("wvT", [D, HPC * HD], FP32R, kind="ExternalInput")
    bv_d = nc.dram_tensor("bv", [P, NPACK], FP32, kind="ExternalInput")
    woT_d = nc.dram_tensor("woT", [NPACK * P, D], FP32R, kind="ExternalInput")
    bo_d = nc.dram_tensor("bo", [P, CT], FP32, kind="ExternalInput")
    if dyn:
        nreps_d = nc.dram_tensor("nreps", [1, 1], mybir.dt.int32,
                                 kind="ExternalInput")
    yT_d = nc.dram_tensor("yT", [D, T], FP32, kind="ExternalOutput")

    with tile.TileContext(nc) as tc:
        with (
            tc.tile_pool(name="persist", bufs=1) as persist,
        ):
            ones_col = nc.const_aps.tensor(1.0, [P, 1], FP32)

            # ---- persistent SBUF residents --------------------------------
            xts = persist.tile([P, CT, T], FP32R, tag="xts")          # 64 KB/p
            vps = persist.tile([P, KT, HPC * (HD + 1)], FP32R, tag="vps")  # 32.5 KB/p
            attnT = persist.tile([P, NPACK, T], FP32R, tag="attnT")   # 32 KB/p
            woTs = persist.tile([P, NPACK, D], FP32R, tag="woTs")     # 16 KB/p
            bqks = persist.tile([P, NPACK, 2], FP32, tag="bqks")
            bvs = persist.tile([P, NPACK], FP32, tag="bvs")
            bos = persist.tile([P, CT], FP32, tag="bos")

            for ct in range(CT):
                nc.sync.dma_start(xts[:, ct, :], xT_d[ct * P:(ct + 1) * P, :])
            nc.sync.dma_start(bqks[:], bqk_d[:, :, :])
            nc.sync.dma_start(bvs[:], bv_d[:, :])
            nc.sync.dma_start(bos[:], bo_d[:, :])
            for ci in range(NPACK):
                nc.sync.dma_start(woTs[:, ci, :], woT_d[ci * P:(ci + 1) * P, :])

            if dyn:
                nrt_sb = persist.tile([1, 1], mybir.dt.int32, tag="nrt")
                nc.sync.dma_start(nrt_sb[:], nreps_d[:, :])
                nval = nc.values_load(nrt_sb[0:1, 0:1], min_val=1,
                                      max_val=1 << 20,
                                      skip_runtime_bounds_check=True)
                rep_ctx = tc.For_i(0, nval, 1)
            else:
                rep_ctx = None

            import contextlib
            with rep_ctx if rep_ctx is not None else contextlib.nullcontext():
              for _ in range(1 if dyn else reps):
                # ---- V generation ----------------------------------------
                with (
                    tc.tile_pool(name="wv_pool", bufs=1) as wv_pool,
                    tc.tile_pool(name="vpsum", bufs=2, space="PSUM") as vpsum,
                ):
                    wvs = wv_pool.tile([P, CT, HPC * HD], FP32R, tag="wvs")
                    for ct in range(CT):
                        nc.sync.dma_start(
                            wvs[:, ct, :], wvT_d[ct * P:(ct + 1) * P, :])
                    # ones columns of V' (col 64 of each head's 65-col block)
                    for tt in range(KT):
                        vview = vps[:, tt, :].rearrange("p (h e) -> p h e", h=HPC)
                        nc.vector.tensor_copy(
                            vview[:, :, HD:HD + 1],
                            ones_col.to_broadcast([P, HPC, 1]))
                    for tt in range(KT):
                        ps = vpsum.tile([P, HPC * HD], FP32, tag="vk")
                        for ct in range(CT):
                            nc.tensor.matmul(
                                ps[:],
                                xts[:, ct, tt * P:(tt + 1) * P],
                                wvs[:, ct, :],
                                start=(ct == 0), stop=(ct == CT - 1))
                        vview = vps[:, tt, :].rearrange("p (h e) -> p h e", h=HPC)
                        nc.vector.tensor_copy(
                            vview[:, :, 0:HD],
                            ps.rearrange("p (h d) -> p h d", h=HPC))

                # ---- per head-pack: QK generation, then attention --------
                with (
                    tc.tile_pool(name="qkt_pool", bufs=1) as qkt_pool,
                    tc.tile_pool(name="wqk_pool", bufs=2) as wqk_pool,
                    tc.tile_pool(name="pt_pool", bufs=4) as pt_pool,
                    tc.tile_pool(name="rep_pool", bufs=2) as rep_pool,
                    tc.tile_pool(name="ystage_pool", bufs=2) as ystage_pool,
                ):
                  for p in range(NPACK):
                    hA, hB = 2 * p, 2 * p + 1

                    wqk = wqk_pool.tile([P, CT, 256], FP32R, tag="wqk")
                    for ct in range(CT):
                        nc.sync.dma_start(
                            wqk[:, ct, :], wqkT_d[ct * P:(ct + 1) * P, p, :])

                    # qkt[:, 0, :] = Q^T pack, qkt[:, 1, :] = K^T pack
                    with tc.tile_pool(name="qkpsum", bufs=2,
                                      space="PSUM") as qkpsum:
                        qkt = qkt_pool.tile([P, 2, T], FP32R, tag="qkt")
                        for jj in range(2):
                            for tb in range(T // 512):
                                ps = qkpsum.tile([P, 512], FP32, tag="qk")
                                for ct in range(CT):
                                    nc.tensor.matmul(
                                        ps[:],
                                        wqk[:, ct, jj * P:(jj + 1) * P],
                                        xts[:, ct, tb * 512:(tb + 1) * 512],
                                        start=(ct == 0), stop=(ct == CT - 1))
                                nc.vector.tensor_scalar_add(
                                    qkt[:, jj, tb * 512:(tb + 1) * 512],
                                    ps[:], bqks[:, p, jj:jj + 1])

                    # ---- attention for this pack -------------------------
                    # Software-pipelined: per kt emit S-matmuls and exps for
                    # kt, then the PV matmuls for kt-1, so the in-order PE
                    # stream never blocks behind an exp the ACT engine is
                    # still computing.
                    with (
                        tc.tile_pool(name="spsum", bufs=2,
                                     space="PSUM") as spsum,
                        tc.tile_pool(name="outpsum", bufs=2,
                                     space="PSUM") as outpsum,
                    ):
                      for qh in range(QH):
                        q0 = qh * QHW
                        # row 64 = softmax denom, row 96 = its reciprocal
                        outA = outpsum.tile([P, QHW], FP32, tag="outp")
                        outB = outpsum.tile([P, QHW], FP32, tag="outp")
                        halves = [(0, HD, outA, hA), (HD, P, outB, hB)]
                        prev = None

                        def emit_pv(entry):
                            ktp, pts = entry
                            for (lo, hi, outp, hloc), pt in zip(halves, pts):
                                for sh in range(QHW // 512):
                                    nc.tensor.matmul(
                                        outp[0:HD + 1, sh * 512:(sh + 1) * 512],
                                        vps[:, ktp,
                                            hloc * (HD + 1):(hloc + 1) * (HD + 1)],
                                        pt[:, sh * 512:(sh + 1) * 512],
                                        start=(ktp == 0), stop=(ktp == KT - 1))

                        for kt in range(KT):
                            pts = []
                            sls = [spsum.tile([P, QHW], FP32, tag="sps",
                                              name=f"sps{h}")
                                   for h in range(2)]
                            # interleave A/B so disjoint row-group matmuls
                            # overlap in the PE array
                            nspass = 2 if variant == "dblmm" else 1
                            for _sp in range(nspass):
                                for sh in range(QHW // 512):
                                    for (lo, hi, outp, hloc), sps in zip(halves, sls):
                                        nc.tensor.matmul(
                                            sps[:, sh * 512:(sh + 1) * 512],
                                            qkt[lo:hi, 1, kt * P:(kt + 1) * P],
                                            qkt[lo:hi, 0,
                                                q0 + sh * 512:q0 + (sh + 1) * 512],
                                            start=True, stop=True)
                            for sps in sls:
                                pt = pt_pool.tile([P, QHW], FP32R, tag="pt")
                                nc.scalar.activation(
                                    pt[:], sps[:], AF.Exp, scale=SCALE)
                                if variant == "dblexp":
                                    nc.scalar.activation(
                                        pt[:], sps[:], AF.Exp, scale=SCALE)
                                pts.append(pt)
                            if prev is not None:
                                emit_pv(prev)
                            prev = (kt, pts)
                        emit_pv(prev)

                        # normalize + v-bias into attnT (A rows 0:64, B 64:128)
                        for row0, outp in [(0, outA), (HD, outB)]:
                            rep = rep_pool.tile([HD, QHW], FP32, tag="rep")
                            nc.vector.reciprocal(
                                rep[0:1, :], outp[HD:HD + 1, :])
                            if variant == "nobcast":
                                nc.vector.memset(rep[:], 1.0)
                            else:
                                nc.gpsimd.partition_broadcast(
                                    rep[:], rep[0:1, :])
                            dst = attnT[row0:row0 + HD, p, q0:q0 + QHW]
                            nc.vector.tensor_tensor(
                                dst, outp[0:HD, :], rep[:], OP.mult)
                            nc.vector.tensor_scalar_add(
                                dst, dst, bvs[row0:row0 + HD, p:p + 1])

                  # ---- out projection ------------------------------------
                  with tc.tile_pool(name="opsum", bufs=4,
                                    space="PSUM") as opsum:
                    for co in range(CT):
                     for tb in range(T // 512):
                        ps = opsum.tile([P, 512], FP32, tag="op")
                        for ci in range(NPACK):
                            nc.tensor.matmul(
                                ps[:],
                                woTs[:, ci, co * P:(co + 1) * P],
                                attnT[:, ci, tb * 512:(tb + 1) * 512],
                                start=(ci == 0), stop=(ci == NPACK - 1))
                        yst = ystage_pool.tile([P, 512], FP32, tag="yst")
                        nc.vector.tensor_scalar_add(
                            yst[:], ps[:], bos[:, co:co + 1])
                        nc.sync.dma_start(
                            yT_d[co * P:(co + 1) * P, tb * 512:(tb + 1) * 512],
                            yst[:])
    nc.compile()
    return nc


def _prep_core_inputs(x, Wqkv, bqkv, Wo, bo, core):
    b, g = core // 2, core % 2
    f32 = np.float32

    xT = np.ascontiguousarray(x[b].T, dtype=f32)

    wqkT = np.empty((D, NPACK, 256), f32)
    bqk = np.empty((P, NPACK, 2), f32)
    for p in range(NPACK):
        rows_q, rows_k = [], []
        for j in range(2):
            h = 8 * g + 2 * p + j
            rows_q.append(slice(192 * h, 192 * h + 64))
            rows_k.append(slice(192 * h + 64, 192 * h + 128))
        Q2 = np.vstack([Wqkv[rows_q[0]], Wqkv[rows_q[1]]])   # [128, D]
        K2 = np.vstack([Wqkv[rows_k[0]], Wqkv[rows_k[1]]])
        wqkT[:, p, :128] = Q2.T
        wqkT[:, p, 128:] = K2.T
        bqk[:, p, 0] = np.concatenate([bqkv[rows_q[0]], bqkv[rows_q[1]]])
        bqk[:, p, 1] = np.concatenate([bqkv[rows_k[0]], bqkv[rows_k[1]]])

    rows_v = [slice(192 * (8 * g + h) + 128, 192 * (8 * g + h) + 192)
              for h in range(HPC)]
    Wv = np.vstack([Wqkv[r] for r in rows_v])                # [512, D]
    wvT = np.ascontiguousarray(Wv.T, dtype=f32)
    bv = np.empty((P, NPACK), f32)
    for p in range(NPACK):
        bv[:64, p] = bqkv[rows_v[2 * p]]
        bv[64:, p] = bqkv[rows_v[2 * p + 1]]

    woT = np.ascontiguousarray(Wo[:, 512 * g:512 * (g + 1)].T, dtype=f32)
    bo2 = (bo.reshape(CT, P).T.astype(f32).copy() if g == 0
           else np.zeros((P, CT), f32))

    return {
        "xT": xT, "wqkT": wqkT, "bqk": bqk, "wvT": wvT,
        "bv": bv, "woT": woT, "bo": bo2,
    }


_NC_CACHE = {}


def kernel(x, Wqkv, bqkv, Wo, bo, _reps: int = 1,
           _return_raw: bool = False):
    x = np.asarray(x, np.float32)
    Wqkv = np.asarray(Wqkv, np.float32)
    bqkv = np.asarray(bqkv, np.float32)
    Wo = np.asarray(Wo, np.float32)
    bo = np.asarray(bo, np.float32)

    in_maps = [_prep_core_inputs(x, Wqkv, bqkv, Wo, bo, c)
               for c in range(N_CORES)]

    if _reps not in _NC_CACHE:
        _NC_CACHE[_reps] = build_nc(_reps)
    nc = _NC_CACHE[_reps]

    res = run_bass_kernel_spmd(nc, in_maps, core_ids=list(range(N_CORES)))
    if _return_raw:
        return res

    y = np.empty((B, T, D), np.float32)
    for b in range(B):
        yt = res.results[2 * b]["yT"] + res.results[2 * b + 1]["yT"]
        y[b] = yt.T
    return y
